# revision 17
# baseline (speedup 1.0000x reference)
"""C2Q (BiDAF-style) attention kernel for 8 TRN2 NeuronCores (v5, HW-tuned).

Pure data parallel: 64 batches sharded 8-per-core. Per batch b (reference):
    S = c @ c_w + (q @ q_w)^T + (c * cq_w) @ q^T + bias      (1024, 128)
    S1 = masked_softmax(S, q_mask, axis=j)
    S2 = masked_softmax(S1, c_mask, axis=i)
    A = S1 @ q ; Bm = S1 @ (S2^T @ c)
    out = [c | A | c*A | c*Bm]                                (1024, 512)

Key algebra: softmax over j is invariant to per-i constants, so c @ c_w
and the bias cancel in S1. Only R[j] = q @ q_w + log-mask(q_mask)
survives as a per-partition bias in the transposed domain.

v2/v3 structure (aimed at the DMA, HWDGE-per-DMA and ACT/DVE rooflines):
  * Device writes ONLY [A | c*A | c*Bm] in fp16 (6MB/core instead of
    16MB f32); the host pastes the exact f32 `c` block and upcasts.
  * c_mask folded multiplicatively: rcprowm = rcprow * m zeroes masked
    rows of S1 BEFORE the second exp, making them exp(0)=1; an exact
    rank-1 correction (ones ⊗ K, K = host-computed masked-row sums of
    [c|1]) is subtracted inside the Traw PSUM accumulation. This keeps
    ONE unmasked cN slab for both the Traw matmul and the elementwise
    c*A / c*Bm products, and drops the per-chunk bias from the G exp.
  * rowsum via 8 one-column PE matmuls (E0T_k^T @ ones).
  * ONE input DMA per batch ([qmod|ones|q|Ts|cT|cN] slab) and ONE
    output DMA per batch (rearranged AP) — HWDGE costs ~625ns per DMA
    instruction, so 19 DMAs/core instead of 91.
  * Stage-pipelined emission: block b emits stA(b+1) | abmm(b-1) woven
    with stB(b+1) | recip_g(b+1) | stC(b) | load(b+2), so the in-order
    PE queue fills ab-rotation waits with next-batch transposes/minis.
  * Output staging (Pool cannot touch PSUM on real HW): per chunk ONE
    psum evacuation with the rcprow scale (DVE, 2 chunks on ACT) writes
    [Bs | As] where As doubles as the A output block; then SBUF-only
    c*A on Pool and c*Bm on DVE (all-f16 4x mode). The out-DMA's 3D AP
    skips the Bs scratch column.

Device per batch:
    S^T[j,i] = qmodT.T @ cT    (f16, 2 matmuls of N=512)
    E0T      = exp(S^T + R[j])               # ACT bias; bf16 [j, 1024]
    ep_k     = transpose(E0T chunk)          # PE; bf16 psum [i, j]
    rowsum_k = E0T_k^T @ ones                # PE minis -> psum f32
    rcprow = 1/rowsum ; rcprowm = rcprow * cmask
    G_k = exp(ep_k * rcprowm_k)              # ACT scale AP; f16 SBUF
    Traw = -ones^T@K + sum_k G_k^T @ [c_k|1] # psum f32 accum [j, 129]
    Ts = Traw[:,0:128] * (1/Traw[:,128])     # -> f16, into slab
    ab_k = E0T_k^T @ [q | Ts]                # psum f32 [i, 256]
    st = [ab_A*r | c*ab_A*r | c*ab_B*r]      # f16, one DMA per batch
No max-subtraction needed: |S+R| <= ~30 so exp stays in range.
"""

import os
import numpy as np
import ml_dtypes

import concourse.bass as bass
import concourse.tile as tile
from concourse import bacc, mybir
from concourse.bass_utils import run_bass_kernel_spmd

F32 = mybir.dt.float32
F16 = mybir.dt.float16
BF16 = mybir.dt.bfloat16
AF = mybir.ActivationFunctionType
ALU = mybir.AluOpType

N_CORES = 8
B, CL, QL, D = 64, 1024, 128, 128
BPC = B // N_CORES          # batches per core
NK = CL // 128              # 128-row chunks per batch
MASK_NEG = -50.0            # exp(-50+eps) vanishes in f32 sums; in ACT range

# input slab column layout (f16)
QMOD0, ONES0, TS0, QROW0, CT0, CN0 = 0, 128, 129, 257, 385, 1409
SLAB = CN0 + NK * 129       # 2441

# staging engine assignment knobs (tuned on HW)
EVAC_ACT = frozenset({6, 7})   # chunk idx whose psum evac runs on ACT (else DVE)
CA_ON_POOL = True              # c*A tensor_mul on Pool (else DVE)
SKIP_IN_DMA = False            # timing probe: skip slab loads
SKIP_OUT_DMA = False           # timing probe: skip out-DMA

LAST_RESULTS = None         # set by kernel() for test.py profiling


def _build_graph(loop_n=0):
    """loop_n=0: straight-line graph (production). loop_n=N>0: wrap the whole
    computation in a hardware For_i loop repeating it N times (timing only)."""
    nc = bacc.Bacc()

    in_ext = nc.declare_dram_parameter("inp", [BPC, 128, SLAB], F16, isOutput=False)
    cmR_ext = nc.declare_dram_parameter("cmR", [128, BPC * NK + BPC], F32, isOutput=False)
    nko_ext = nc.declare_dram_parameter("nko", [1, BPC * 129 + 128], F16, isOutput=False)
    id_ext = nc.declare_dram_parameter("ident", [128, 128], BF16, isOutput=False)
    out_ext = nc.declare_dram_parameter("out", [BPC, CL, 3 * D], F16, isOutput=True)

    with tile.TileContext(nc) as tc:
        with (
            tc.tile_pool(name="const", bufs=1) as const,
            tc.tile_pool(name="inp", bufs=8) as inp,
            tc.tile_pool(name="e0tp", bufs=5) as e0tp,
            tc.tile_pool(name="gp", bufs=4) as gp,
            tc.tile_pool(name="small", bufs=6) as smallp,
            tc.tile_pool(name="stg", bufs=4) as stg,
            tc.tile_pool(name="stp", bufs=1, space=bass.MemorySpace.PSUM) as stp,
            tc.tile_pool(name="epp", bufs=2, space=bass.MemorySpace.PSUM) as epp,
            tc.tile_pool(name="rsp", bufs=1, space=bass.MemorySpace.PSUM) as rsp,
            tc.tile_pool(name="trawp", bufs=1, space=bass.MemorySpace.PSUM) as trawp,
            tc.tile_pool(name="abp", bufs=3, space=bass.MemorySpace.PSUM) as abp,
        ):
            ident = const.tile([128, 128], BF16, tag="ident")
            nc.sync.dma_start(ident[:], id_ext[:])
            cmR = const.tile([128, BPC * NK + BPC], F32, tag="cmR")
            nc.sync.dma_start(cmR[:], cmR_ext[:])
            nko = const.tile([1, BPC * 129 + 128], F16, tag="nko")
            nc.sync.dma_start(nko[:], nko_ext[:])

            IN = {}
            E0T = {}
            EP = {}
            RS = {}
            G = {}
            RCP = {}
            ST = {}
            AB2 = {}

            def load(b):
                t = inp.tile([128, SLAB], F16, tag="in")
                if not SKIP_IN_DMA:
                    nc.sync.dma_start(t[:], in_ext[b])
                IN[b] = t

            def stA(b):
                """S^T + E0T exp."""
                t = IN[b]
                e0t = e0tp.tile([128, CL], BF16, tag="e0t")
                for h in range(2):
                    sp = stp.tile([128, 512], F32, tag="sp")
                    nc.tensor.matmul(
                        sp[:], t[:, QMOD0:QMOD0 + 128],
                        t[:, CT0 + h * 512:CT0 + (h + 1) * 512],
                    )
                    nc.scalar.activation(
                        e0t[:, h * 512:(h + 1) * 512], sp[:], AF.Exp,
                        bias=cmR[:, BPC * NK + b:BPC * NK + b + 1],
                    )
                E0T[b] = e0t

            def transp(b, k):
                if k == 0:
                    EP[b] = epp.tile([128, CL], BF16, tag="ep", name="ep")
                nc.tensor.transpose(
                    EP[b][:, k * 128:(k + 1) * 128],
                    E0T[b][:, k * 128:(k + 1) * 128], ident[:],
                )

            def mini(b, k):
                if k == 0:
                    RS[b] = rsp.tile([128, NK], F32, tag="rs", name="rs")
                nc.tensor.matmul(
                    RS[b][:, k:k + 1],
                    E0T[b][:, k * 128:(k + 1) * 128], IN[b][:, ONES0:ONES0 + 1],
                )

            def recip_g(b):
                """rcprow(+mask) and the 8 G exps."""
                rcprow = smallp.tile([128, NK], F32, tag="rcprow")
                rcprowm = smallp.tile([128, NK], F32, tag="rcprowm")
                nc.vector.reciprocal_approx_fast(rcprow[:], RS[b][:])
                nc.vector.tensor_mul(
                    rcprowm[:], rcprow[:], cmR[:, b * NK:(b + 1) * NK]
                )
                g = gp.tile([128, CL], F16, tag="g")
                for k in range(NK):
                    nc.scalar.activation(
                        g[:, k * 128:(k + 1) * 128],
                        EP[b][:, k * 128:(k + 1) * 128], AF.Exp,
                        scale=rcprowm[:, k:k + 1],
                    )
                G[b], RCP[b] = g, rcprow

            def stC(b):
                """Traw accumulation (rank-1 mask fix first), then Ts."""
                t, g = IN[b], G[b]
                traw = trawp.tile([128, 129], F32, tag="traw", name="traw")[:]
                nc.tensor.matmul(
                    traw, nko[0:1, BPC * 129:BPC * 129 + 128],
                    nko[0:1, b * 129:(b + 1) * 129],
                    start=True, stop=False,
                )
                for k in range(NK):
                    nc.tensor.matmul(
                        traw, g[:, k * 128:(k + 1) * 128],
                        t[:, CN0 + k * 129:CN0 + (k + 1) * 129],
                        start=False, stop=(k == NK - 1),
                    )
                rcp2 = smallp.tile([128, 1], F32, tag="rcp2")
                nc.vector.reciprocal_approx_fast(rcp2[:], traw[:, 128:129])
                nc.vector.tensor_scalar_mul(
                    t[:, TS0:TS0 + 128], traw[:, 0:128], rcp2[:]
                )

            def abmm(b, k):
                """One AB matmul + staging; batched out-DMA on the last.

                ab = [Braw | Araw] (Ts precedes q in the slab). Staging per
                chunk: ONE psum evacuation with the rcprow scale (DVE/ACT;
                Pool cannot touch PSUM) -> st [Bs | As]; As doubles as the
                output A block. Then SBUF-only products c*A (Pool) and
                c*Bm (DVE 4x). Out-DMA skips the Bs scratch column.
                chunk st layout: [Bs | A | c*A | c*Bm] (512 cols)."""
                t, e0t, rcprow = IN[b], E0T[b], RCP[b]
                if k == 0:
                    ST[b] = stg.tile([128, NK * 512], F16, tag="st", name="st")
                st = ST[b]
                ab = abp.tile([128, 2 * QL], F32, tag="ab", name="ab")[:]
                nc.tensor.matmul(
                    ab, e0t[:, k * 128:(k + 1) * 128],
                    t[:, TS0:TS0 + 256],
                )
                s0 = k * 512
                cchunk = t[:, CN0 + k * 129:CN0 + k * 129 + 128]
                # evac psum with scale: st[Bs|As] = ab * rcprow_k
                evac_act = k in EVAC_ACT or (b == BPC - 1 and k % 2 == 1)
                if not evac_act:
                    nc.vector.tensor_scalar_mul(
                        st[:, s0:s0 + 256], ab, rcprow[:, k:k + 1]
                    )
                else:
                    nc.scalar.activation(
                        st[:, s0:s0 + 256], ab, AF.Copy,
                        scale=rcprow[:, k:k + 1],
                    )
                # c*A = As * c  (SBUF-only; Pool cannot touch PSUM but can this)
                ca_eng = nc.gpsimd if CA_ON_POOL else nc.vector
                ca_eng.tensor_mul(
                    st[:, s0 + 256:s0 + 384], st[:, s0 + 128:s0 + 256], cchunk
                )
                # c*Bm = Bs * c  (DVE, all-SBUF f16 -> 4x mode)
                nc.vector.tensor_mul(
                    st[:, s0 + 384:s0 + 512], st[:, s0:s0 + 128], cchunk
                )
                if k == NK - 1 and not SKIP_OUT_DMA:
                    nc.sync.dma_start(
                        out_ext[b].rearrange("(k p) d -> p k d", p=128),
                        st[:].rearrange("p (k d) -> p k d", d=512)[:, :, 128:512],
                    )

            def run_all():
                # pipeline: block b emits stA(b+1) | abmm(b-1)⊗stB(b+1) |
                # recip_g(b+1) | stC(b) | load(b+2). In-order PE queue fills
                # ab-rotation and exp(b+1) waits with interleaved work.
                load(0)
                load(1)
                stA(0)
                for k in range(NK):
                    transp(0, k)
                    mini(0, k)
                recip_g(0)
                for b in range(BPC):
                    if b + 1 < BPC:
                        stA(b + 1)
                    for k in range(NK):
                        if b >= 1:
                            abmm(b - 1, k)
                        if b + 1 < BPC:
                            transp(b + 1, k)
                            mini(b + 1, k)
                    if b + 1 < BPC:
                        recip_g(b + 1)
                    stC(b)
                    if b + 2 < BPC:
                        load(b + 2)
                for k in range(NK):
                    abmm(BPC - 1, k)

            if loop_n:
                with tc.For_i(0, loop_n, 1):
                    run_all()
            else:
                run_all()
    return nc


def _prep(c, q, c_mask, q_mask, c_weight, q_weight, cq_weight, bias):
    c = np.ascontiguousarray(np.asarray(c, dtype=np.float32))
    q = np.ascontiguousarray(np.asarray(q, dtype=np.float32))
    c_mask = np.asarray(c_mask)
    q_mask = np.asarray(q_mask)
    q_weight = np.asarray(q_weight, dtype=np.float32)
    cq_weight = np.asarray(cq_weight, dtype=np.float32)
    f16 = np.float16

    # host-side prep (tiny). NOTE: c@c_weight and bias cancel in softmax_j.
    s1 = (q.reshape(-1, D) @ q_weight).reshape(B, QL)          # (B, 128)
    R = s1 + np.where(q_mask > 0, 0.0, MASK_NEG).astype(np.float32)
    cm = (c_mask > 0).astype(np.float32)                       # (B, 1024)

    cT = c.transpose(0, 2, 1).astype(f16)                      # (B, 128, 1024)
    qmodT = (q * cq_weight.reshape(1, 1, D)).transpose(0, 2, 1).astype(f16)
    # cN: natural chunks [c_k | 1] -> (B, 128, NK*129)
    cNc = c.reshape(B, NK, 128, D).transpose(0, 2, 1, 3)       # (B, p, k, d)
    cN = np.concatenate(
        [cNc, np.ones((B, 128, NK, 1), np.float32)], axis=3
    ).astype(f16).reshape(B, 128, NK * 129)
    slab = np.concatenate(
        [qmodT, np.ones((B, 128, 1), f16), np.zeros((B, 128, 128), f16),
         q.astype(f16), cT, cN], axis=2
    )                                                          # (B, 128, SLAB)
    # rank-1 mask correction: K = [sum_masked c | n_masked] per batch
    w = 1.0 - cm                                               # masked rows
    Kc = np.einsum('bi,bid->bd', w, c)                         # (B, 128)
    Kn = w.sum(axis=1)                                         # (B,)
    negK = -np.concatenate([Kc, Kn[:, None]], axis=1)          # (B, 129)

    in_maps = []
    for core in range(N_CORES):
        sl = slice(core * BPC, (core + 1) * BPC)
        cmN = cm[sl].reshape(BPC, NK, 128).transpose(2, 0, 1).reshape(128, BPC * NK)
        cmR = np.ascontiguousarray(
            np.concatenate([cmN, R[sl].T], axis=1)             # (128, 64+8)
        )
        nko = np.concatenate(
            [negK[sl].reshape(1, BPC * 129), np.ones((1, 128), np.float32)],
            axis=1,
        ).astype(f16)
        in_maps.append({
            "inp": np.ascontiguousarray(slab[sl]),
            "cmR": cmR,
            "nko": np.ascontiguousarray(nko),
            "ident": np.eye(128, dtype=ml_dtypes.bfloat16),
        })
    return in_maps


def make_in_maps():
    """For the local test/compare harness only (imports reference)."""
    import reference
    inputs = {k: np.asarray(v) for k, v in reference.setup_inputs().items()}
    return _prep(**inputs)


def _assemble(c, dev_out):
    """dev_out: (B, CL, 384) f16 -> full (B, CL, 512) f32 with exact c block."""
    out = np.empty((B, CL, 4 * D), dtype=np.float32)
    out[:, :, 0:D] = c
    out[:, :, D:] = dev_out.astype(np.float32)
    return out


def kernel(c, q, c_mask, q_mask, c_weight, q_weight, cq_weight, bias):
    global LAST_RESULTS
    c = np.ascontiguousarray(np.asarray(c, dtype=np.float32))
    in_maps = _prep(c, q, c_mask, q_mask, c_weight, q_weight, cq_weight, bias)
    os.environ["BASS_NEVER_TRACE"] = "1"  # no NTFF hook in this container
    nc = _build_graph()
    nc.finalize()
    res = run_bass_kernel_spmd(nc, in_maps, core_ids=list(range(N_CORES)))
    LAST_RESULTS = (nc, in_maps)
    dev = np.concatenate([res.results[i]["out"] for i in range(N_CORES)], axis=0)
    return _assemble(c, dev)


# revision 20
# speedup vs baseline: 1.0683x; 1.0683x over previous
"""C2Q (BiDAF-style) attention kernel for 8 TRN2 NeuronCores (v5, HW-tuned).

Pure data parallel: 64 batches sharded 8-per-core. Per batch b (reference):
    S = c @ c_w + (q @ q_w)^T + (c * cq_w) @ q^T + bias      (1024, 128)
    S1 = masked_softmax(S, q_mask, axis=j)
    S2 = masked_softmax(S1, c_mask, axis=i)
    A = S1 @ q ; Bm = S1 @ (S2^T @ c)
    out = [c | A | c*A | c*Bm]                                (1024, 512)

Key algebra: softmax over j is invariant to per-i constants, so c @ c_w
and the bias cancel in S1. Only R[j] = q @ q_w + log-mask(q_mask)
survives as a per-partition bias in the transposed domain.

v2/v3 structure (aimed at the DMA, HWDGE-per-DMA and ACT/DVE rooflines):
  * Device writes ONLY [A | c*A | c*Bm] in fp16 (6MB/core instead of
    16MB f32); the host pastes the exact f32 `c` block and upcasts.
  * c_mask folded multiplicatively: rcprowm = rcprow * m zeroes masked
    rows of S1 BEFORE the second exp, making them exp(0)=1; an exact
    rank-1 correction (ones ⊗ K, K = host-computed masked-row sums of
    [c|1]) is subtracted inside the Traw PSUM accumulation. This keeps
    ONE unmasked cN slab for both the Traw matmul and the elementwise
    c*A / c*Bm products, and drops the per-chunk bias from the G exp.
  * rowsum via 8 one-column PE matmuls (E0T_k^T @ ones).
  * ONE input DMA per batch ([qmod|ones|q|Ts|cT|cN] slab) and ONE
    output DMA per batch (rearranged AP) — HWDGE costs ~625ns per DMA
    instruction, so 19 DMAs/core instead of 91.
  * Stage-pipelined emission: block b emits stA(b+1) | abmm(b-1) woven
    with stB(b+1) | recip_g(b+1) | stC(b) | load(b+2), so the in-order
    PE queue fills ab-rotation waits with next-batch transposes/minis.
  * Output staging (Pool cannot touch PSUM on real HW): per chunk ONE
    psum evacuation with the rcprow scale (DVE, 2 chunks on ACT) writes
    [Bs | As] where As doubles as the A output block; then SBUF-only
    c*A on Pool and c*Bm on DVE (all-f16 4x mode). The out-DMA's 3D AP
    skips the Bs scratch column.

Device per batch:
    S^T[j,i] = qmodT.T @ cT    (f16, 2 matmuls of N=512)
    E0T      = exp(S^T + R[j])               # ACT bias; bf16 [j, 1024]
    ep_k     = transpose(E0T chunk)          # PE; bf16 psum [i, j]
    rowsum_k = E0T_k^T @ ones                # PE minis -> psum f32
    rcprow = 1/rowsum ; rcprowm = rcprow * cmask
    G_k = exp(ep_k * rcprowm_k)              # ACT scale AP; f16 SBUF
    Traw = -ones^T@K + sum_k G_k^T @ [c_k|1] # psum f32 accum [j, 129]
    Ts = Traw[:,0:128] * (1/Traw[:,128])     # -> f16, into slab
    ab_k = E0T_k^T @ [q | Ts]                # psum f32 [i, 256]
    st = [ab_A*r | c*ab_A*r | c*ab_B*r]      # f16, one DMA per batch
No max-subtraction needed: |S+R| <= ~30 so exp stays in range.
"""

import os
import numpy as np
import ml_dtypes

import concourse.bass as bass
import concourse.tile as tile
from concourse import bacc, mybir
from concourse.bass_utils import run_bass_kernel_spmd

F32 = mybir.dt.float32
F16 = mybir.dt.float16
BF16 = mybir.dt.bfloat16
AF = mybir.ActivationFunctionType
ALU = mybir.AluOpType

N_CORES = 8
B, CL, QL, D = 64, 1024, 128, 128
BPC = B // N_CORES          # batches per core
NK = CL // 128              # 128-row chunks per batch
MASK_NEG = -50.0            # exp(-50+eps) vanishes in f32 sums; in ACT range

# input slab column layout (f16)
QMOD0, ONES0, TS0, QROW0, CT0, CN0 = 0, 128, 129, 257, 385, 1409
SLAB = CN0 + NK * 129       # 2441

# staging engine assignment knobs (tuned on HW)
EVAC_ACT = frozenset({6, 7})   # chunk idx whose psum evac runs on ACT (else DVE)
CA_ON_POOL = True              # c*A tensor_mul on Pool (else DVE)
SKIP_IN_DMA = False            # timing probe: skip slab loads
SKIP_OUT_DMA = False           # timing probe: skip out-DMA
NBATCH = BPC                   # timing probe: process only first N batches
STAGGER = True                 # For_i staggered reset (overlap loop iterations)

LAST_RESULTS = None         # set by kernel() for test.py profiling


def _build_graph(loop_n=0):
    """loop_n=0: straight-line graph (production). loop_n=N>0: wrap the whole
    computation in a hardware For_i loop repeating it N times (timing only)."""
    nc = bacc.Bacc()

    in_ext = nc.declare_dram_parameter("inp", [BPC, 128, SLAB], F16, isOutput=False)
    cmR_ext = nc.declare_dram_parameter("cmR", [128, BPC * NK + BPC], F32, isOutput=False)
    nko_ext = nc.declare_dram_parameter("nko", [1, BPC * 129 + 128], F16, isOutput=False)
    id_ext = nc.declare_dram_parameter("ident", [128, 128], BF16, isOutput=False)
    out_ext = nc.declare_dram_parameter("out", [BPC, CL, 3 * D], F16, isOutput=True)

    with tile.TileContext(nc) as tc:
        with (
            tc.tile_pool(name="const", bufs=1) as const,
            tc.tile_pool(name="inp", bufs=8) as inp,
            tc.tile_pool(name="e0tp", bufs=5) as e0tp,
            tc.tile_pool(name="gp", bufs=4) as gp,
            tc.tile_pool(name="small", bufs=6) as smallp,
            tc.tile_pool(name="stg", bufs=4) as stg,
            tc.tile_pool(name="stp", bufs=1, space=bass.MemorySpace.PSUM) as stp,
            tc.tile_pool(name="epp", bufs=2, space=bass.MemorySpace.PSUM) as epp,
            tc.tile_pool(name="rsp", bufs=1, space=bass.MemorySpace.PSUM) as rsp,
            tc.tile_pool(name="trawp", bufs=1, space=bass.MemorySpace.PSUM) as trawp,
            tc.tile_pool(name="abp", bufs=2, space=bass.MemorySpace.PSUM) as abp,
        ):
            ident = const.tile([128, 128], BF16, tag="ident")
            nc.sync.dma_start(ident[:], id_ext[:])
            cmR = const.tile([128, BPC * NK + BPC], F32, tag="cmR")
            nc.sync.dma_start(cmR[:], cmR_ext[:])
            nko = const.tile([1, BPC * 129 + 128], F16, tag="nko")
            nc.sync.dma_start(nko[:], nko_ext[:])

            IN = {}
            E0T = {}
            EP = {}
            RS = {}
            G = {}
            RCP = {}
            ST = {}
            AB2 = {}

            def load(b):
                t = inp.tile([128, SLAB], F16, tag="in")
                if not SKIP_IN_DMA:
                    nc.sync.dma_start(t[:, 0:CN0], in_ext[b][:, 0:CN0])
                    nc.sync.dma_start(t[:, CN0:SLAB], in_ext[b][:, CN0:SLAB])
                IN[b] = t

            def stA(b):
                """S^T + E0T exp."""
                t = IN[b]
                e0t = e0tp.tile([128, CL], BF16, tag="e0t")
                sp = stp.tile([128, CL], F32, tag="sp")
                for h in range(2):
                    nc.tensor.matmul(
                        sp[:, h * 512:(h + 1) * 512], t[:, QMOD0:QMOD0 + 128],
                        t[:, CT0 + h * 512:CT0 + (h + 1) * 512],
                    )
                nc.scalar.activation(
                    e0t[:], sp[:], AF.Exp,
                    bias=cmR[:, BPC * NK + b:BPC * NK + b + 1],
                )
                E0T[b] = e0t

            def transp(b, k):
                if k == 0:
                    EP[b] = epp.tile([128, CL], BF16, tag="ep", name="ep")
                nc.tensor.transpose(
                    EP[b][:, k * 128:(k + 1) * 128],
                    E0T[b][:, k * 128:(k + 1) * 128], ident[:],
                )

            def mini(b, k):
                if k == 0:
                    RS[b] = rsp.tile([128, NK], F32, tag="rs", name="rs")
                nc.tensor.matmul(
                    RS[b][:, k:k + 1],
                    E0T[b][:, k * 128:(k + 1) * 128], IN[b][:, ONES0:ONES0 + 1],
                )

            def recip_g(b):
                """rcprow(+mask) and the 8 G exps."""
                rcprow = smallp.tile([128, NK], F32, tag="rcprow")
                rcprowm = smallp.tile([128, NK], F32, tag="rcprowm")
                nc.vector.reciprocal_approx_fast(rcprow[:], RS[b][:])
                nc.vector.tensor_mul(
                    rcprowm[:], rcprow[:], cmR[:, b * NK:(b + 1) * NK]
                )
                g = gp.tile([128, CL], F16, tag="g")
                for k in range(NK):
                    nc.scalar.activation(
                        g[:, k * 128:(k + 1) * 128],
                        EP[b][:, k * 128:(k + 1) * 128], AF.Exp,
                        scale=rcprowm[:, k:k + 1],
                    )
                G[b], RCP[b] = g, rcprow

            def stC(b):
                """Traw accumulation (rank-1 mask fix first), then Ts."""
                t, g = IN[b], G[b]
                traw = trawp.tile([128, 129], F32, tag="traw", name="traw")[:]
                nc.tensor.matmul(
                    traw, nko[0:1, BPC * 129:BPC * 129 + 128],
                    nko[0:1, b * 129:(b + 1) * 129],
                    start=True, stop=False,
                )
                for k in range(NK):
                    nc.tensor.matmul(
                        traw, g[:, k * 128:(k + 1) * 128],
                        t[:, CN0 + k * 129:CN0 + (k + 1) * 129],
                        start=False, stop=(k == NK - 1),
                    )
                rcp2 = smallp.tile([128, 1], F32, tag="rcp2")
                nc.vector.reciprocal_approx_fast(rcp2[:], traw[:, 128:129])
                nc.vector.tensor_scalar_mul(
                    t[:, TS0:TS0 + 128], traw[:, 0:128], rcp2[:]
                )

            def abmm(b, k):
                """One AB matmul + staging; batched out-DMA on the last.

                ab = [Braw | Araw] (Ts precedes q in the slab). Staging per
                chunk: ONE psum evacuation with the rcprow scale (DVE/ACT;
                Pool cannot touch PSUM) -> st [Bs | As]; As doubles as the
                output A block. Then SBUF-only products c*A (Pool) and
                c*Bm (DVE 4x). Out-DMA skips the Bs scratch column.
                chunk st layout: [Bs | A | c*A | c*Bm] (512 cols)."""
                t, e0t, rcprow = IN[b], E0T[b], RCP[b]
                if k == 0:
                    ST[b] = stg.tile([128, NK * 512], F16, tag="st", name="st")
                st = ST[b]
                ab = abp.tile([128, 2 * QL], F32, tag="ab", name="ab")[:]
                nc.tensor.matmul(
                    ab, e0t[:, k * 128:(k + 1) * 128],
                    t[:, TS0:TS0 + 256],
                )
                s0 = k * 512
                cchunk = t[:, CN0 + k * 129:CN0 + k * 129 + 128]
                # evac psum with scale: st[Bs|As] = ab * rcprow_k
                evac_act = k in EVAC_ACT or (b == BPC - 1 and k % 2 == 1)
                if not evac_act:
                    nc.vector.tensor_scalar_mul(
                        st[:, s0:s0 + 256], ab, rcprow[:, k:k + 1]
                    )
                else:
                    nc.scalar.activation(
                        st[:, s0:s0 + 256], ab, AF.Copy,
                        scale=rcprow[:, k:k + 1],
                    )
                # c*A = As * c  (SBUF-only; Pool cannot touch PSUM but can this)
                ca_eng = nc.gpsimd if CA_ON_POOL else nc.vector
                ca_eng.tensor_mul(
                    st[:, s0 + 256:s0 + 384], st[:, s0 + 128:s0 + 256], cchunk
                )
                # c*Bm = Bs * c  (DVE, all-SBUF f16 -> 4x mode)
                nc.vector.tensor_mul(
                    st[:, s0 + 384:s0 + 512], st[:, s0:s0 + 128], cchunk
                )
                if k in (NK // 2 - 1, NK - 1) and not SKIP_OUT_DMA:
                    half = 0 if k == NK // 2 - 1 else 1
                    h0 = half * (NK // 2) * 128
                    nc.sync.dma_start(
                        out_ext[b][h0 * 4:(h0 + 512) * 4 // 4 + h0 * 3].rearrange(
                            "(k p) d -> p k d", p=128)
                        if False else
                        out_ext[b, half * 512:(half + 1) * 512, :].rearrange(
                            "(k p) d -> p k d", p=128),
                        st[:].rearrange("p (k d) -> p k d", d=512)[
                            :, half * (NK // 2):(half + 1) * (NK // 2), 128:512],
                    )

            def run_all():
                # pipeline: block b emits stA(b+1) | abmm(b-1)⊗stB(b+1) |
                # recip_g(b+1) | stC(b) | load(b+2). In-order PE queue fills
                # ab-rotation and exp(b+1) waits with interleaved work.
                NB = NBATCH
                load(0)
                if NB > 1:
                    load(1)
                stA(0)
                for k in range(NK):
                    transp(0, k)
                    mini(0, k)
                recip_g(0)
                for b in range(NB):
                    if b + 1 < NB:
                        stA(b + 1)
                    for k in range(NK):
                        if b >= 1:
                            abmm(b - 1, k)
                        if b + 1 < NB:
                            transp(b + 1, k)
                            mini(b + 1, k)
                    if b + 1 < NB:
                        recip_g(b + 1)
                    stC(b)
                    if b + 2 < NB:
                        load(b + 2)
                for k in range(NK):
                    abmm(NB - 1, k)

            if loop_n:
                with tc.For_i(0, loop_n, 1, staggered_reset=STAGGER):
                    run_all()
            else:
                run_all()
    return nc


def _prep(c, q, c_mask, q_mask, c_weight, q_weight, cq_weight, bias):
    c = np.ascontiguousarray(np.asarray(c, dtype=np.float32))
    q = np.ascontiguousarray(np.asarray(q, dtype=np.float32))
    c_mask = np.asarray(c_mask)
    q_mask = np.asarray(q_mask)
    q_weight = np.asarray(q_weight, dtype=np.float32)
    cq_weight = np.asarray(cq_weight, dtype=np.float32)
    f16 = np.float16

    # host-side prep (tiny). NOTE: c@c_weight and bias cancel in softmax_j.
    s1 = (q.reshape(-1, D) @ q_weight).reshape(B, QL)          # (B, 128)
    R = s1 + np.where(q_mask > 0, 0.0, MASK_NEG).astype(np.float32)
    cm = (c_mask > 0).astype(np.float32)                       # (B, 1024)

    cT = c.transpose(0, 2, 1).astype(f16)                      # (B, 128, 1024)
    qmodT = (q * cq_weight.reshape(1, 1, D)).transpose(0, 2, 1).astype(f16)
    # cN: natural chunks [c_k | 1] -> (B, 128, NK*129)
    cNc = c.reshape(B, NK, 128, D).transpose(0, 2, 1, 3)       # (B, p, k, d)
    cN = np.concatenate(
        [cNc, np.ones((B, 128, NK, 1), np.float32)], axis=3
    ).astype(f16).reshape(B, 128, NK * 129)
    slab = np.concatenate(
        [qmodT, np.ones((B, 128, 1), f16), np.zeros((B, 128, 128), f16),
         q.astype(f16), cT, cN], axis=2
    )                                                          # (B, 128, SLAB)
    # rank-1 mask correction: K = [sum_masked c | n_masked] per batch
    w = 1.0 - cm                                               # masked rows
    Kc = np.einsum('bi,bid->bd', w, c)                         # (B, 128)
    Kn = w.sum(axis=1)                                         # (B,)
    negK = -np.concatenate([Kc, Kn[:, None]], axis=1)          # (B, 129)

    in_maps = []
    for core in range(N_CORES):
        sl = slice(core * BPC, (core + 1) * BPC)
        cmN = cm[sl].reshape(BPC, NK, 128).transpose(2, 0, 1).reshape(128, BPC * NK)
        cmR = np.ascontiguousarray(
            np.concatenate([cmN, R[sl].T], axis=1)             # (128, 64+8)
        )
        nko = np.concatenate(
            [negK[sl].reshape(1, BPC * 129), np.ones((1, 128), np.float32)],
            axis=1,
        ).astype(f16)
        in_maps.append({
            "inp": np.ascontiguousarray(slab[sl]),
            "cmR": cmR,
            "nko": np.ascontiguousarray(nko),
            "ident": np.eye(128, dtype=ml_dtypes.bfloat16),
        })
    return in_maps


def make_in_maps():
    """For the local test/compare harness only (imports reference)."""
    import reference
    inputs = {k: np.asarray(v) for k, v in reference.setup_inputs().items()}
    return _prep(**inputs)


def _assemble(c, dev_out):
    """dev_out: (B, CL, 384) f16 -> full (B, CL, 512) f32 with exact c block."""
    out = np.empty((B, CL, 4 * D), dtype=np.float32)
    out[:, :, 0:D] = c
    out[:, :, D:] = dev_out.astype(np.float32)
    return out


def kernel(c, q, c_mask, q_mask, c_weight, q_weight, cq_weight, bias):
    global LAST_RESULTS
    c = np.ascontiguousarray(np.asarray(c, dtype=np.float32))
    in_maps = _prep(c, q, c_mask, q_mask, c_weight, q_weight, cq_weight, bias)
    os.environ["BASS_NEVER_TRACE"] = "1"  # no NTFF hook in this container
    nc = _build_graph()
    nc.finalize()
    res = run_bass_kernel_spmd(nc, in_maps, core_ids=list(range(N_CORES)))
    LAST_RESULTS = (nc, in_maps)
    dev = np.concatenate([res.results[i]["out"] for i in range(N_CORES)], axis=0)
    return _assemble(c, dev)


# revision 21
# speedup vs baseline: 1.0766x; 1.0077x over previous
"""C2Q (BiDAF-style) attention kernel for 8 TRN2 NeuronCores (v5, HW-tuned).

Pure data parallel: 64 batches sharded 8-per-core. Per batch b (reference):
    S = c @ c_w + (q @ q_w)^T + (c * cq_w) @ q^T + bias      (1024, 128)
    S1 = masked_softmax(S, q_mask, axis=j)
    S2 = masked_softmax(S1, c_mask, axis=i)
    A = S1 @ q ; Bm = S1 @ (S2^T @ c)
    out = [c | A | c*A | c*Bm]                                (1024, 512)

Key algebra: softmax over j is invariant to per-i constants, so c @ c_w
and the bias cancel in S1. Only R[j] = q @ q_w + log-mask(q_mask)
survives as a per-partition bias in the transposed domain.

v2/v3 structure (aimed at the DMA, HWDGE-per-DMA and ACT/DVE rooflines):
  * Device writes ONLY [A | c*A | c*Bm] in fp16 (6MB/core instead of
    16MB f32); the host pastes the exact f32 `c` block and upcasts.
  * c_mask folded multiplicatively: rcprowm = rcprow * m zeroes masked
    rows of S1 BEFORE the second exp, making them exp(0)=1; an exact
    rank-1 correction (ones ⊗ K, K = host-computed masked-row sums of
    [c|1]) is subtracted inside the Traw PSUM accumulation. This keeps
    ONE unmasked cN slab for both the Traw matmul and the elementwise
    c*A / c*Bm products, and drops the per-chunk bias from the G exp.
  * rowsum via 8 one-column PE matmuls (E0T_k^T @ ones).
  * ONE input DMA per batch ([qmod|ones|q|Ts|cT|cN] slab) and ONE
    output DMA per batch (rearranged AP) — HWDGE costs ~625ns per DMA
    instruction, so 19 DMAs/core instead of 91.
  * Stage-pipelined emission: block b emits stA(b+1) | abmm(b-1) woven
    with stB(b+1) | recip_g(b+1) | stC(b) | load(b+2), so the in-order
    PE queue fills ab-rotation waits with next-batch transposes/minis.
  * Output staging (Pool cannot touch PSUM on real HW): per chunk ONE
    psum evacuation with the rcprow scale (DVE, 2 chunks on ACT) writes
    [Bs | As] where As doubles as the A output block; then SBUF-only
    c*A on Pool and c*Bm on DVE (all-f16 4x mode). The out-DMA's 3D AP
    skips the Bs scratch column.

Device per batch:
    S^T[j,i] = qmodT.T @ cT    (f16, 2 matmuls of N=512)
    E0T      = exp(S^T + R[j])               # ACT bias; bf16 [j, 1024]
    ep_k     = transpose(E0T chunk)          # PE; bf16 psum [i, j]
    rowsum_k = E0T_k^T @ ones                # PE minis -> psum f32
    rcprow = 1/rowsum ; rcprowm = rcprow * cmask
    G_k = exp(ep_k * rcprowm_k)              # ACT scale AP; f16 SBUF
    Traw = -ones^T@K + sum_k G_k^T @ [c_k|1] # psum f32 accum [j, 129]
    Ts = Traw[:,0:128] * (1/Traw[:,128])     # -> f16, into slab
    ab_k = E0T_k^T @ [q | Ts]                # psum f32 [i, 256]
    st = [ab_A*r | c*ab_A*r | c*ab_B*r]      # f16, one DMA per batch
No max-subtraction needed: |S+R| <= ~30 so exp stays in range.
"""

import os
import numpy as np
import ml_dtypes

import concourse.bass as bass
import concourse.tile as tile
from concourse import bacc, mybir
from concourse.bass_utils import run_bass_kernel_spmd

F32 = mybir.dt.float32
F16 = mybir.dt.float16
BF16 = mybir.dt.bfloat16
AF = mybir.ActivationFunctionType
ALU = mybir.AluOpType

N_CORES = 8
B, CL, QL, D = 64, 1024, 128, 128
BPC = B // N_CORES          # batches per core
NK = CL // 128              # 128-row chunks per batch
MASK_NEG = -50.0            # exp(-50+eps) vanishes in f32 sums; in ACT range

# input slab column layout (f16)
QMOD0, ONES0, TS0, QROW0, CT0, CN0 = 0, 128, 129, 257, 385, 1409
SLAB = CN0 + NK * 129       # 2441

# staging engine assignment knobs (tuned on HW)
EVAC_ACT = frozenset({6, 7})   # chunk idx whose psum evac runs on ACT (else DVE)
CA_ON_POOL = True              # c*A tensor_mul on Pool (else DVE)
SKIP_IN_DMA = False            # timing probe: skip slab loads
SKIP_OUT_DMA = False           # timing probe: skip out-DMA
NBATCH = BPC                   # timing probe: process only first N batches
STAGGER = True                 # For_i staggered reset (overlap loop iterations)

LAST_RESULTS = None         # set by kernel() for test.py profiling


def _build_graph(loop_n=0):
    """loop_n=0: straight-line graph (production). loop_n=N>0: wrap the whole
    computation in a hardware For_i loop repeating it N times (timing only)."""
    nc = bacc.Bacc()

    in_ext = nc.declare_dram_parameter("inp", [BPC, 128, SLAB], F16, isOutput=False)
    cmR_ext = nc.declare_dram_parameter("cmR", [128, BPC * NK + BPC], F32, isOutput=False)
    nko_ext = nc.declare_dram_parameter("nko", [1, BPC * 129 + 128], F16, isOutput=False)
    id_ext = nc.declare_dram_parameter("ident", [128, 128], BF16, isOutput=False)
    out_ext = nc.declare_dram_parameter("out", [BPC, CL, 3 * D], F16, isOutput=True)

    with tile.TileContext(nc) as tc:
        with (
            tc.tile_pool(name="const", bufs=1) as const,
            tc.tile_pool(name="inp", bufs=8) as inp,
            tc.tile_pool(name="e0tp", bufs=5) as e0tp,
            tc.tile_pool(name="gp", bufs=4) as gp,
            tc.tile_pool(name="small", bufs=6) as smallp,
            tc.tile_pool(name="stg", bufs=4) as stg,
            tc.tile_pool(name="stp", bufs=1, space=bass.MemorySpace.PSUM) as stp,
            tc.tile_pool(name="epp", bufs=2, space=bass.MemorySpace.PSUM) as epp,
            tc.tile_pool(name="rsp", bufs=1, space=bass.MemorySpace.PSUM) as rsp,
            tc.tile_pool(name="trawp", bufs=1, space=bass.MemorySpace.PSUM) as trawp,
            tc.tile_pool(name="abp", bufs=2, space=bass.MemorySpace.PSUM) as abp,
        ):
            ident = const.tile([128, 128], BF16, tag="ident")
            nc.sync.dma_start(ident[:], id_ext[:])
            cmR = const.tile([128, BPC * NK + BPC], F32, tag="cmR")
            nc.sync.dma_start(cmR[:], cmR_ext[:])
            nko = const.tile([1, BPC * 129 + 128], F16, tag="nko")
            nc.sync.dma_start(nko[:], nko_ext[:])

            IN = {}
            E0T = {}
            EP = {}
            RS = {}
            G = {}
            RCP = {}
            ST = {}
            AB2 = {}

            def load(b):
                t = inp.tile([128, SLAB], F16, tag="in")
                if not SKIP_IN_DMA:
                    nc.sync.dma_start(t[:], in_ext[b])
                IN[b] = t

            def stA(b):
                """S^T + E0T exp."""
                t = IN[b]
                e0t = e0tp.tile([128, CL], BF16, tag="e0t")
                sp = stp.tile([128, CL], F32, tag="sp")
                for h in range(2):
                    nc.tensor.matmul(
                        sp[:, h * 512:(h + 1) * 512], t[:, QMOD0:QMOD0 + 128],
                        t[:, CT0 + h * 512:CT0 + (h + 1) * 512],
                    )
                nc.scalar.activation(
                    e0t[:], sp[:], AF.Exp,
                    bias=cmR[:, BPC * NK + b:BPC * NK + b + 1],
                )
                E0T[b] = e0t

            def transp(b, k):
                if k == 0:
                    EP[b] = epp.tile([128, CL], BF16, tag="ep", name="ep")
                nc.tensor.transpose(
                    EP[b][:, k * 128:(k + 1) * 128],
                    E0T[b][:, k * 128:(k + 1) * 128], ident[:],
                )

            def mini(b, k):
                if k == 0:
                    RS[b] = rsp.tile([128, NK], F32, tag="rs", name="rs")
                nc.tensor.matmul(
                    RS[b][:, k:k + 1],
                    E0T[b][:, k * 128:(k + 1) * 128], IN[b][:, ONES0:ONES0 + 1],
                )

            def recip_g(b):
                """rcprow(+mask) and the 8 G exps."""
                rcprow = smallp.tile([128, NK], F32, tag="rcprow")
                rcprowm = smallp.tile([128, NK], F32, tag="rcprowm")
                nc.vector.reciprocal_approx_fast(rcprow[:], RS[b][:])
                nc.vector.tensor_mul(
                    rcprowm[:], rcprow[:], cmR[:, b * NK:(b + 1) * NK]
                )
                g = gp.tile([128, CL], F16, tag="g")
                for k in range(NK):
                    nc.scalar.activation(
                        g[:, k * 128:(k + 1) * 128],
                        EP[b][:, k * 128:(k + 1) * 128], AF.Exp,
                        scale=rcprowm[:, k:k + 1],
                    )
                G[b], RCP[b] = g, rcprow

            def stC(b):
                """Traw accumulation (rank-1 mask fix first), then Ts."""
                t, g = IN[b], G[b]
                traw = trawp.tile([128, 129], F32, tag="traw", name="traw")[:]
                nc.tensor.matmul(
                    traw, nko[0:1, BPC * 129:BPC * 129 + 128],
                    nko[0:1, b * 129:(b + 1) * 129],
                    start=True, stop=False,
                )
                for k in range(NK):
                    nc.tensor.matmul(
                        traw, g[:, k * 128:(k + 1) * 128],
                        t[:, CN0 + k * 129:CN0 + (k + 1) * 129],
                        start=False, stop=(k == NK - 1),
                    )
                rcp2 = smallp.tile([128, 1], F32, tag="rcp2")
                nc.vector.reciprocal_approx_fast(rcp2[:], traw[:, 128:129])
                nc.vector.tensor_scalar_mul(
                    t[:, TS0:TS0 + 128], traw[:, 0:128], rcp2[:]
                )

            def abmm(b, k):
                """One AB matmul + staging; batched out-DMA on the last.

                ab = [Braw | Araw] (Ts precedes q in the slab). Staging per
                chunk: ONE psum evacuation with the rcprow scale (DVE/ACT;
                Pool cannot touch PSUM) -> st [Bs | As]; As doubles as the
                output A block. Then SBUF-only products c*A (Pool) and
                c*Bm (DVE 4x). Out-DMA skips the Bs scratch column.
                chunk st layout: [Bs | A | c*A | c*Bm] (512 cols)."""
                t, e0t, rcprow = IN[b], E0T[b], RCP[b]
                if k == 0:
                    ST[b] = stg.tile([128, NK * 512], F16, tag="st", name="st")
                st = ST[b]
                ab = abp.tile([128, 2 * QL], F32, tag="ab", name="ab")[:]
                nc.tensor.matmul(
                    ab, e0t[:, k * 128:(k + 1) * 128],
                    t[:, TS0:TS0 + 256],
                )
                s0 = k * 512
                cchunk = t[:, CN0 + k * 129:CN0 + k * 129 + 128]
                # evac psum with scale: st[Bs|As] = ab * rcprow_k
                evac_act = k in EVAC_ACT or (b == BPC - 1 and k % 2 == 1)
                if not evac_act:
                    nc.vector.tensor_scalar_mul(
                        st[:, s0:s0 + 256], ab, rcprow[:, k:k + 1]
                    )
                else:
                    nc.scalar.activation(
                        st[:, s0:s0 + 256], ab, AF.Copy,
                        scale=rcprow[:, k:k + 1],
                    )
                # c*A = As * c  (SBUF-only; Pool cannot touch PSUM but can this)
                ca_eng = nc.gpsimd if CA_ON_POOL else nc.vector
                ca_eng.tensor_mul(
                    st[:, s0 + 256:s0 + 384], st[:, s0 + 128:s0 + 256], cchunk
                )
                # c*Bm = Bs * c  (DVE, all-SBUF f16 -> 4x mode)
                nc.vector.tensor_mul(
                    st[:, s0 + 384:s0 + 512], st[:, s0:s0 + 128], cchunk
                )
                if k == NK - 1 and not SKIP_OUT_DMA:
                    nc.sync.dma_start(
                        out_ext[b].rearrange("(k p) d -> p k d", p=128),
                        st[:].rearrange("p (k d) -> p k d", d=512)[:, :, 128:512],
                    )

            def run_all():
                # pipeline: block b emits stA(b+1) | abmm(b-1)⊗stB(b+1) |
                # recip_g(b+1) | stC(b) | load(b+2). In-order PE queue fills
                # ab-rotation and exp(b+1) waits with interleaved work.
                NB = NBATCH
                load(0)
                if NB > 1:
                    load(1)
                stA(0)
                for k in range(NK):
                    transp(0, k)
                    mini(0, k)
                recip_g(0)
                for b in range(NB):
                    if b + 1 < NB:
                        stA(b + 1)
                    for k in range(NK):
                        if b >= 1:
                            abmm(b - 1, k)
                        if b + 1 < NB:
                            transp(b + 1, k)
                            mini(b + 1, k)
                    if b + 1 < NB:
                        recip_g(b + 1)
                    stC(b)
                    if b + 2 < NB:
                        load(b + 2)
                for k in range(NK):
                    abmm(NB - 1, k)

            if loop_n:
                with tc.For_i(0, loop_n, 1, staggered_reset=STAGGER):
                    run_all()
            else:
                run_all()
    return nc


def _prep(c, q, c_mask, q_mask, c_weight, q_weight, cq_weight, bias):
    c = np.ascontiguousarray(np.asarray(c, dtype=np.float32))
    q = np.ascontiguousarray(np.asarray(q, dtype=np.float32))
    c_mask = np.asarray(c_mask)
    q_mask = np.asarray(q_mask)
    q_weight = np.asarray(q_weight, dtype=np.float32)
    cq_weight = np.asarray(cq_weight, dtype=np.float32)
    f16 = np.float16

    # host-side prep (tiny). NOTE: c@c_weight and bias cancel in softmax_j.
    s1 = (q.reshape(-1, D) @ q_weight).reshape(B, QL)          # (B, 128)
    R = s1 + np.where(q_mask > 0, 0.0, MASK_NEG).astype(np.float32)
    cm = (c_mask > 0).astype(np.float32)                       # (B, 1024)

    cT = c.transpose(0, 2, 1).astype(f16)                      # (B, 128, 1024)
    qmodT = (q * cq_weight.reshape(1, 1, D)).transpose(0, 2, 1).astype(f16)
    # cN: natural chunks [c_k | 1] -> (B, 128, NK*129)
    cNc = c.reshape(B, NK, 128, D).transpose(0, 2, 1, 3)       # (B, p, k, d)
    cN = np.concatenate(
        [cNc, np.ones((B, 128, NK, 1), np.float32)], axis=3
    ).astype(f16).reshape(B, 128, NK * 129)
    slab = np.concatenate(
        [qmodT, np.ones((B, 128, 1), f16), np.zeros((B, 128, 128), f16),
         q.astype(f16), cT, cN], axis=2
    )                                                          # (B, 128, SLAB)
    # rank-1 mask correction: K = [sum_masked c | n_masked] per batch
    w = 1.0 - cm                                               # masked rows
    Kc = np.einsum('bi,bid->bd', w, c)                         # (B, 128)
    Kn = w.sum(axis=1)                                         # (B,)
    negK = -np.concatenate([Kc, Kn[:, None]], axis=1)          # (B, 129)

    in_maps = []
    for core in range(N_CORES):
        sl = slice(core * BPC, (core + 1) * BPC)
        cmN = cm[sl].reshape(BPC, NK, 128).transpose(2, 0, 1).reshape(128, BPC * NK)
        cmR = np.ascontiguousarray(
            np.concatenate([cmN, R[sl].T], axis=1)             # (128, 64+8)
        )
        nko = np.concatenate(
            [negK[sl].reshape(1, BPC * 129), np.ones((1, 128), np.float32)],
            axis=1,
        ).astype(f16)
        in_maps.append({
            "inp": np.ascontiguousarray(slab[sl]),
            "cmR": cmR,
            "nko": np.ascontiguousarray(nko),
            "ident": np.eye(128, dtype=ml_dtypes.bfloat16),
        })
    return in_maps


def make_in_maps():
    """For the local test/compare harness only (imports reference)."""
    import reference
    inputs = {k: np.asarray(v) for k, v in reference.setup_inputs().items()}
    return _prep(**inputs)


def _assemble(c, dev_out):
    """dev_out: (B, CL, 384) f16 -> full (B, CL, 512) f32 with exact c block."""
    out = np.empty((B, CL, 4 * D), dtype=np.float32)
    out[:, :, 0:D] = c
    out[:, :, D:] = dev_out.astype(np.float32)
    return out


def kernel(c, q, c_mask, q_mask, c_weight, q_weight, cq_weight, bias):
    global LAST_RESULTS
    c = np.ascontiguousarray(np.asarray(c, dtype=np.float32))
    in_maps = _prep(c, q, c_mask, q_mask, c_weight, q_weight, cq_weight, bias)
    os.environ["BASS_NEVER_TRACE"] = "1"  # no NTFF hook in this container
    nc = _build_graph()
    nc.finalize()
    res = run_bass_kernel_spmd(nc, in_maps, core_ids=list(range(N_CORES)))
    LAST_RESULTS = (nc, in_maps)
    dev = np.concatenate([res.results[i]["out"] for i in range(N_CORES)], axis=0)
    return _assemble(c, dev)
